# revision 39
# baseline (speedup 1.0000x reference)
"""Multi-head attention on 8 Trainium2 NeuronCores.

Problem: B=2, S=2048, D=1024, H=16 heads (head_dim 64), boolean mask,
per-head gate, QKV/out linear projections.

Sharding: core c handles batch b=c//4 and heads 4*(c%4)..4*(c%4)+3.
Each core computes its 4 heads' attention and the partial output
projection; the host sums the 4 partials per batch and adds the constant
terms (bo, and the bv/gate contribution which is constant because
attention rows sum to 1).

Design (v2): the attention core (scores/exp/mask/PV/normalize/O-proj)
stays fp16 as in v1, but the QKV projections run as 3-term fp8e4m3
DoubleRow matmul chains:  x@W ~= x8@W8 + x8@Wlo + xlo@W8, where
x = x8 + xlo and 32*W = W8 + Wlo are host-prepared residual splits
(W scaled x32 so the residual stays in fp8 normal range; the evacuation
rescales by 1/32).  DoubleRow contracts 256 rows per pass at 0.5
cycles/row, so each projection costs 0.75x its fp16 cycles while adding
<0.2% error (measured end-to-end rel err ~2.3e-3 vs the 2e-2 gate).
fp8 x tensors also halve the input DMA bytes, pulling the first exp to
~7us.  The ACT engine runs the 128 x [128,1024] exp stream exclusively;
all psum evacuations live on DVE (tensor_scalar mult+bias).  concat is
built 16x-scaled (ones column = 1/16) so nothing changes on-chip; the
host divides the od/odx partials by 16.
"""

import sys

if "/opt/trn_rl_repo" not in sys.path:
    sys.path.insert(0, "/opt/trn_rl_repo")

import numpy as np
import ml_dtypes

import concourse.bass as bass
import concourse.bacc as bacc
import concourse.mybir as mybir
import concourse.tile as tile
from concourse.bass_utils import run_bass_kernel_spmd

BF16 = mybir.dt.float16  # fp16: same speed as bf16, 3 more mantissa bits
F32 = mybir.dt.float32
FP8 = mybir.dt.float8e4
NPBF16 = np.float16
NPFP8 = ml_dtypes.float8_e4m3
DR = mybir.MatmulPerfMode.DoubleRow
MULT = mybir.AluOpType.mult
ADD = mybir.AluOpType.add

P = 128
B, S, D = 2, 2048, 1024
HEADS, HD = 16, 64
NCORES = 8
NH = HEADS // (NCORES // B)  # heads per core = 4
COLS = NH * HD               # 256 concat columns per core
NPAIR = D // 256             # 4 DoubleRow contraction pairs (256 rows each)
SKT = S // P                 # 16 key chunks
SQB = 1024                   # query block width in the attention loop
NSQB = S // SQB
WSC = 1.0 / 32.0             # evac rescale (W is host-scaled x32)

_CACHE = {}


def _build_program():
    nc = bacc.Bacc("TRN2", debug=False)

    # x tensors in partition-major DoubleRow pair layout:
    # [D,S] -> [128, 4 pairs, 2, S] flattened to [128, 8S], so each
    # tensor-variant is ONE contiguous 128-descriptor DMA.
    xdram = {}
    for t in ("q", "k", "v"):
        for v_ in ("8", "l"):
            xdram[(t, v_)] = nc.declare_dram_parameter(
                f"x{t}{v_}", [P, NPAIR * 2 * S], FP8, isOutput=False
            )
    # weights in pair layout [128, 4 pairs, 2, 256] -> [128, 2048]
    # (partition-major so one DMA is 128 contiguous descriptors)
    wdram = {}
    for t in ("q", "k", "v"):
        for v_ in ("8", "l"):
            wdram[(t, v_)] = nc.declare_dram_parameter(
                f"w{t}{v_}", [P, NPAIR * 2 * COLS], FP8, isOutput=False
            )
    mT = nc.declare_dram_parameter("mT", [S, S], BF16, isOutput=False)
    wo = nc.declare_dram_parameter("wo", [COLS, D], BF16, isOutput=False)
    biasf = nc.declare_dram_parameter("biasf", [P, 4], F32, isOutput=False)
    od = nc.declare_dram_parameter("od", [D, S], BF16, isOutput=True)
    # cc1 (heads 2,3) partial of the LAST sq block, summed on host
    odx = nc.declare_dram_parameter("odx", [D, SQB], BF16, isOutput=True)

    xdr3 = {k: v[:] for k, v in xdram.items()}
    wdr3 = {k: v[:] for k, v in wdram.items()}
    mT3 = mT[:].rearrange("(n p) s -> n p s", p=P)
    wo3 = wo[:].rearrange("(n p) d -> n p d", p=P)
    wo3b = wo[:].rearrange("(n p) d -> p n d", p=P)
    od3 = od[:].rearrange("(n p) s -> n p s", p=P)
    odx3 = odx[:].rearrange("(n p) s -> n p s", p=P)

    def dr2(ap):
        # [p, 2*n] tile view -> [p, 2, n] for DoubleRow operands
        return ap.rearrange("p (two n) -> p two n", two=2)

    with tile.TileContext(nc) as tc:
        with (
            tc.tile_pool(name="wpool", bufs=1) as wpool,
            tc.tile_pool(name="qkpool", bufs=1) as qkpool,
            tc.tile_pool(name="vpool", bufs=1) as vpool,
            tc.tile_pool(name="maskpool", bufs=1) as maskpool,
            tc.tile_pool(name="cpool", bufs=1) as cpool,
            tc.tile_pool(name="xpool", bufs=1) as xpool,
            tc.tile_pool(name="pmpool", bufs=1) as pmpool,
            tc.tile_pool(name="npool", bufs=1) as npool,
            tc.tile_pool(name="opool", bufs=1) as opool,
        ):
            # ---------------- DMA issue (t=0) ----------------------------
            # Every DMA's transfer serializes on its issuing queue at
            # ~0.385ns/byte/partition (+ ~1us fixed), so the ramp is pure
            # pipe scheduling.  x tensors arrive column-sliced (K per
            # 512-slice, Q/V per S-half) so the first exp only waits for
            # ~36KB/partition instead of the full 96KB.
            w_sb = {}
            xparts = {}

            def wdma(t, v_, eng):
                tl = wpool.tile([P, NPAIR, 2 * COLS], FP8, name=f"w{t}{v_}")
                eng.dma_start(out=tl[:], in_=wdr3[(t, v_)])
                w_sb[(t, v_)] = [dr2(tl[:, i, :]) for i in range(NPAIR)]

            def xdma(t, v_, c0, c1, eng):
                # logical x columns [c0:c1) of all pairs, as its own tile
                w = c1 - c0
                tl = xpool.tile(
                    [P, NPAIR, 2, w], FP8, name=f"x{t}{v_}", tag=f"x{t}{v_}{c0}",
                    bufs=1,
                )
                eng.dma_start(
                    out=tl[:],
                    in_=xdr3[(t, v_)].rearrange(
                        "p (n two s) -> p n two s", n=NPAIR, two=2
                    )[:, :, :, c0:c1],
                )
                xparts.setdefault((t, v_), []).append((tl, c0, c1))

            def xap(t, v_, i, c0, c1):
                for tl, a_, b_ in xparts[(t, v_)]:
                    if a_ <= c0 and c1 <= b_:
                        return tl[:, i, :, c0 - a_ : c1 - a_]
                raise KeyError((t, v_, c0, c1))

            # mask pairs: [128, 2 skc, SQB] fp16 tiles, one DMA each
            mT2 = mT[:].rearrange("(j u p) s -> j p u s", u=2, p=P)

            def mask_dma(j, sqb, eng):
                t = maskpool.tile(
                    [P, 2, SQB], BF16, name=f"mp{j}", tag=f"mp{j}", bufs=1
                )
                eng.dma_start(
                    out=t[:], in_=mT2[j][:, :, sqb * SQB : (sqb + 1) * SQB]
                )
                return t

            # SP pipe
            xdma("k", "8", 0, 512, nc.sync)
            wdma("q", "8", nc.sync)
            wdma("q", "l", nc.sync)
            bias_t = wpool.tile([P, 4], F32, name="bias_t")
            nc.sync.dma_start(out=bias_t[:], in_=biasf[:])
            b_sb = {
                ("bk", 0): bias_t[:, 0:1],
                ("bk", 1): bias_t[:, 1:2],
                ("bq", 0): bias_t[:, 2:3],
                ("bq", 1): bias_t[:, 3:4],
            }
            xdma("q", "l", 0, SQB, nc.sync)
            mp_sb = [None] * 8
            mp_sb[1] = mask_dma(1, 0, nc.sync)
            xdma("k", "8", 512, 1024, nc.sync)
            mp_sb[2] = mask_dma(2, 0, nc.sync)
            xdma("k", "8", 1024, 1536, nc.sync)
            xdma("k", "8", 1536, 2048, nc.sync)
            xdma("v", "l", 0, SQB, nc.sync)
            mp_sb[4] = mask_dma(4, 0, nc.sync)
            xdma("v", "l", SQB, S, nc.sync)
            mp_sb[6] = mask_dma(6, 0, nc.sync)
            xdma("q", "8", SQB, S, nc.sync)
            xdma("q", "l", SQB, S, nc.sync)
            # ACT pipe (the implicit act-table load runs first; it warms
            # the exp table)
            wdma("k", "8", nc.scalar)
            wdma("k", "l", nc.scalar)
            xdma("q", "8", 0, SQB, nc.scalar)
            mp_sb[0] = mask_dma(0, 0, nc.scalar)
            # Pool pipe
            xdma("k", "l", 0, 512, nc.gpsimd)
            xdma("k", "l", 512, 1024, nc.gpsimd)
            xdma("k", "l", 1024, 1536, nc.gpsimd)
            xdma("k", "l", 1536, 2048, nc.gpsimd)
            wdma("v", "8", nc.gpsimd)
            wdma("v", "l", nc.gpsimd)
            mp_sb[3] = mask_dma(3, 0, nc.gpsimd)
            xdma("v", "8", 0, SQB, nc.gpsimd)
            mp_sb[5] = mask_dma(5, 0, nc.gpsimd)
            xdma("v", "8", SQB, S, nc.gpsimd)
            mp_sb[7] = mask_dma(7, 0, nc.gpsimd)
            m_sb = [mp_sb[i // 2][:, i % 2, :] for i in range(SKT)]

            # wo follows on SP (needed at ~85us)
            wo_all = wpool.tile([P, COLS // P, D], BF16, name="wo_all")
            nc.sync.dma_start(out=wo_all[:], in_=wo3b)
            wo_sb = [wo_all[:, i, :] for i in range(COLS // P)]

            # concat^T (normalized attention outputs, 16x-scaled, head-major)
            concat_sb = [
                cpool.tile([P, S], BF16, name=f"concat_sb{i}")
                for i in range(COLS // P)
            ]

            # ---------------- projections + attention --------------------
            # qh/kh are built from [128,512] S-slice pieces, each a
            # self-contained 12-matmul 3-term DoubleRow chain + DVE evac
            # (fp16 qh = psum/32 + bias).  Only K slice 0 and Q slices 0/1
            # run before attention; everything else (incl. the c1 halves
            # for heads 2/3) drains as pending pieces into h0/h1 slack.
            qhT_sb = {}
            for tname in ("k", "q"):
                for ch in range(2):
                    qhT_sb[(tname, ch)] = qkpool.tile(
                        [P, S], BF16, name=f"{tname}hT{ch}"
                    )

            def chain_terms(t):
                return (
                    (w_sb[(t, "8")], "8"),
                    (w_sb[(t, "l")], "8"),
                    (w_sb[(t, "8")], "l"),
                )

            vh_sb = [None] * SKT

            with (
                tc.tile_pool(name="ps_s", bufs=1, space="PSUM") as ps_s_pool,
                tc.tile_pool(name="ps_pv", bufs=1, space="PSUM") as ps_pv_pool,
            ):
                def emit_vproj(skt):
                    # one V-proj piece: vh[skt] = (xv @ Wv)[skt block]/32,
                    # 3-term DR chain; ones column = 1/16 (16x concat scale)
                    psv = ps_pv_pool.tile(
                        [P, 512], F32, name="psv", tag="scratch", bufs=2
                    )
                    n = 0
                    for w_list, xv_ in chain_terms("v"):
                        for i in range(NPAIR):
                            nc.tensor.matmul(
                                psv[:, 0:COLS],
                                lhsT=xap("v", xv_, i, skt * P, (skt + 1) * P),
                                rhs=w_list[i][:],
                                start=(n == 0),
                                stop=(n == 3 * NPAIR - 1),
                                perf_mode=DR,
                            )
                            n += 1
                    vt = vpool.tile([P, NH, HD + 1], BF16, name=f"vh_sb{skt}")
                    nc.vector.tensor_scalar(
                        vt[:, :, 0:HD],
                        psv[:, 0:COLS].rearrange("p (h d) -> p h d", h=NH),
                        WSC, None, MULT,
                    )
                    nc.gpsimd.memset(vt[:, :, HD], 1.0 / 16.0)
                    vh_sb[skt] = vt

                def emit_oproj_split(sqb, dc, half, dest3, dq0):
                    # the two cc-matmuls of an O-proj piece as separate
                    # closures so one drains per slot (keeps every sqb1
                    # slot's PE load above the exp period)
                    q0 = sqb * SQB + half * 512
                    cell = {}

                    def part0():
                        po = ps_pv_pool.tile(
                            [P, 512], F32, name="po", tag="scratch", bufs=2
                        )
                        cell["po"] = po
                        nc.tensor.matmul(
                            po[:],
                            lhsT=wo_sb[0][:, dc * P : (dc + 1) * P],
                            rhs=concat_sb[0][:, q0 : q0 + 512],
                            start=True,
                            stop=False,
                        )

                    def part1():
                        po = cell["po"]
                        nc.tensor.matmul(
                            po[:],
                            lhsT=wo_sb[1][:, dc * P : (dc + 1) * P],
                            rhs=concat_sb[1][:, q0 : q0 + 512],
                            start=False,
                            stop=True,
                        )
                        oev = opool.tile([P, 512], BF16, name="oev", tag="oev", bufs=6)
                        nc.vector.tensor_copy(oev[:], po[:])
                        nc.sync.dma_start(
                            out=dest3[dc][:, dq0 + half * 512 : dq0 + half * 512 + 512],
                            in_=oev[:],
                        )

                    return part0, part1

                def emit_oproj_piece(sqb, dc, half, ccs, dest3, dq0, evac="pool", dma=None, pool=None, rows=None):
                    # po[128d, 512sq] = sum_cc wo_cc^T @ concat_cc  (fp16)
                    q0 = sqb * SQB + half * 512
                    r0_, r1_ = rows if rows is not None else (0, P)
                    if pool == "pss":
                        po_w = ps_s_pool.tile([P, SQB], F32, name="pss", tag="pss", bufs=2)
                        po = po_w[:, 0:512]
                    else:
                        po = ps_pv_pool.tile(
                            [P, 512], F32, name="po", tag="scratch", bufs=2
                        )[:]
                    for j, cc in enumerate(ccs):
                        nc.tensor.matmul(
                            po[:],
                            lhsT=wo_sb[cc][r0_:r1_, dc * P : (dc + 1) * P],
                            rhs=concat_sb[cc][r0_:r1_, q0 : q0 + 512],
                            start=(j == 0),
                            stop=(j == len(ccs) - 1),
                        )
                    oev = opool.tile([P, 512], BF16, name="oev", tag="oev", bufs=6)
                    if evac == "act":
                        nc.scalar.copy(oev[:], po)
                    else:
                        nc.vector.tensor_copy(oev[:], po)
                    (dma or nc.sync).dma_start(
                        out=dest3[dc][:, dq0 + half * 512 : dq0 + half * 512 + 512],
                        in_=oev[:],
                    )

                ones64 = npool.tile([HD + 1, HD], BF16, name="ones64", tag="ones64", bufs=1)
                nc.vector.memset(ones64[:], 1.0)

                # deferred PE pieces, drained one per skc iteration.
                pending = []

                def drain(n):
                    k = 0
                    while pending and k < n:
                        fn, needs_concat = pending[0]
                        if needs_concat and carry_norm:
                            break
                        pending.pop(0)
                        fn()
                        k += 1

                def emit_proj_piece(tname, ch, sb):
                    # one [128,512] S-slice of a K/Q projection half: a
                    # 12-matmul 3-term DR chain into a scratch psum slot
                    pst = ps_pv_pool.tile(
                        [P, 512], F32, name="pc1", tag="scratch", bufs=2
                    )
                    n = 0
                    for w_list, xv_ in chain_terms(tname):
                        for i in range(NPAIR):
                            nc.tensor.matmul(
                                pst[:],
                                lhsT=w_list[i][:, :, ch * P : (ch + 1) * P],
                                rhs=xap(tname, xv_, i, sb * 512, (sb + 1) * 512),
                                start=(n == 0),
                                stop=(n == 3 * NPAIR - 1),
                                perf_mode=DR,
                            )
                            n += 1
                    nc.vector.tensor_scalar(
                        qhT_sb[(tname, ch)][:, sb * 512 : (sb + 1) * 512],
                        pst[:], WSC, b_sb[(f"b{tname}", ch)], MULT, ADD,
                    )

                # ramp: K slice 0 + Q slices 0/1 inline, rest pending in
                # first-use order (h0 reads kT0 skc-progressively, h2 at
                # slot 32 reads qT1/kT1, sqb1 from slot 64 reads qT0 s2/3)
                emit_proj_piece("k", 0, 0)
                emit_proj_piece("q", 0, 0)
                emit_proj_piece("q", 0, 1)
                for tname, ch, sb in (
                    ("k", 0, 1), ("k", 0, 2), ("k", 0, 3),
                    ("q", 1, 0), ("q", 1, 1),
                    ("k", 1, 0), ("k", 1, 1), ("k", 1, 2), ("k", 1, 3),
                    ("q", 0, 2), ("q", 0, 3),
                    ("q", 1, 2), ("q", 1, 3),
                ):
                    pending.append(
                        (lambda t=tname, c=ch, s=sb: emit_proj_piece(t, c, s), False)
                    )

                def make_norm(pv, h, ht, q0, fast=False):
                    # normalization for head h's pv accumulator, cut into 5
                    # pieces spread over the next head's early iterations.
                    # denom row 64 = sum(pm)/16 -> recip gives the 16x
                    # concat scale for free.
                    dnc = npool.tile(
                        [HD + 1, SQB], BF16, name="dnc", tag="dnc", bufs=2
                    )

                    def evac_half(i):
                        def fn():
                            nc.vector.tensor_copy(
                                dnc[:, i * 512 : (i + 1) * 512], pv[i][:]
                            )
                        return fn

                    def evac():
                        evac_half(0)()
                        evac_half(1)()

                    rbs = {}

                    def recip_piece(half):
                        def fn():
                            sl = slice(half * 512, half * 512 + 512)
                            if fast:
                                with nc.allow_low_precision("fp16 softmax denom"):
                                    nc.vector.reciprocal(
                                        out=dnc[HD : HD + 1, sl],
                                        in_=pv[half][HD : HD + 1, :],
                                    )
                                rb = ps_pv_pool.tile(
                                    [P, 512], F32, name="rbf", tag="scratch", bufs=2
                                )
                                nc.tensor.matmul(
                                    rb[0:HD, :],
                                    lhsT=ones64[HD : HD + 1, :],
                                    rhs=dnc[HD : HD + 1, sl],
                                    start=True,
                                    stop=True,
                                )
                                rbs[half] = rb[0:HD, :]
                                return
                            dn0 = npool.tile([1, 512], BF16, name="dn0", tag="dn0", bufs=1)
                            nc.sync.dma_start(out=dn0[:], in_=dnc[HD : HD + 1, sl])
                            with nc.allow_low_precision("fp16 softmax denom"):
                                nc.vector.reciprocal(out=dn0[:], in_=dn0[:])
                            rb = npool.tile(
                                [HD, 512], BF16, name=f"rb{half}", tag=f"rb{half}", bufs=1
                            )
                            nc.gpsimd.partition_broadcast(rb[:], dn0[:])
                            rbs[half] = rb[:]
                        return fn

                    def mul_piece(half):
                        def fn():
                            sl = slice(half * 512, half * 512 + 512)
                            cs = slice(q0 + half * 512, q0 + half * 512 + 512)
                            if h % 2 == 0:
                                nc.vector.tensor_mul(
                                    concat_sb[ht][0:HD, cs], dnc[0:HD, sl], rbs[half]
                                )
                            else:
                                tmp = npool.tile([HD, 512], BF16, name="tmpn", tag="tmpn", bufs=1)
                                nc.vector.tensor_mul(tmp[:], dnc[0:HD, sl], rbs[half])
                                nc.sync.dma_start(out=concat_sb[ht][HD:P, cs], in_=tmp[:])
                        return fn

                    if fast:
                        return [
                            evac_half(0), recip_piece(0), mul_piece(0),
                            evac_half(1), recip_piece(1), mul_piece(1),
                        ]
                    return [evac, recip_piece(0), mul_piece(0), recip_piece(1), mul_piece(1)]

                carry_pv = []
                carry_norm = []

                m_cur = m_sb
                m_next = [None] * SKT
                for sqb in range(NSQB):
                    q0 = sqb * SQB
                    last_sqb = sqb == NSQB - 1
                    if sqb == 1:
                        m_cur = m_next
                    heads = [0, 1, 3, 2] if last_sqb else [0, 1, 2, 3]
                    for hi, h in enumerate(heads):
                        ht, hp = h // 2, HD * (h % 2)
                        qT = qhT_sb[("q", ht)]
                        kT = qhT_sb[("k", ht)]
                        pv = [
                            ps_pv_pool.tile(
                                [HD + 1, 512], F32, name=f"pspv{i}", tag=f"pspv{i}", bufs=1
                            )
                            for i in range(2)
                        ]

                        def emit_pv(pm_t, skc, pv=pv, h=h):
                            for i in range(SQB // 512):
                                nc.tensor.matmul(
                                    pv[i][:],
                                    lhsT=vh_sb[skc][:, h, :],
                                    rhs=pm_t[:, i * 512 : (i + 1) * 512],
                                    start=(skc == 0),
                                    stop=(skc == SKT - 1),
                                )

                        # h0 of sqb0 also hosts the V-proj pieces (shifted
                        # 5 slots late: the xv DMAs land ~14-21us); its PV
                        # trails by 7; V pieces 11-15 run early in h1
                        trail = 7 if (sqb == 0 and hi == 0) else 2
                        pms = [None] * SKT
                        for skc in range(SKT):
                            ss = ps_s_pool.tile(
                                [P, SQB], F32, name="pss", tag="pss", bufs=2
                            )
                            for i in range(SQB // 512):
                                nc.tensor.matmul(
                                    ss[:, i * 512 : (i + 1) * 512],
                                    lhsT=kT[hp : hp + HD, skc * P : (skc + 1) * P],
                                    rhs=qT[hp : hp + HD, q0 + i * 512 : q0 + (i + 1) * 512],
                                    start=True,
                                    stop=True,
                                )
                            if skc <= 6 and carry_pv:
                                carry_pv.pop(0)()
                            if sqb == 0 and hi == 0:
                                if skc >= 5:
                                    emit_vproj(skc - 5)
                                if skc % 4 == 1:
                                    drain(1)
                            else:
                                if sqb == 0 and hi == 1 and skc <= 4:
                                    emit_vproj(11 + skc)
                                if (
                                    (skc % 4 == 1 if sqb == 0 else skc % 2 == 1)
                                    or (last_sqb and hi >= 2 and len(pending) >= 1)
                                ):
                                    drain(1)
                            if skc >= trail:
                                emit_pv(pms[skc - trail], skc - trail)
                            pm = pmpool.tile([P, SQB], BF16, name="pm", tag="pm", bufs=8)
                            if last_sqb and hi == NH - 1 and skc == SKT - 1:
                                for eh in range(2):
                                    es = slice(eh * 512, eh * 512 + 512)
                                    nc.scalar.activation(
                                        pm[:, es], ss[:, es],
                                        mybir.ActivationFunctionType.Exp,
                                    )
                            else:
                                nc.scalar.activation(
                                    pm[:], ss[:], mybir.ActivationFunctionType.Exp
                                )
                            meng = nc.gpsimd if skc % 8 in (2, 5, 7) else nc.vector
                            meng.tensor_mul(pm[:], pm[:], m_cur[skc])
                            pms[skc] = pm
                            if (sqb == 0 and hi == NH - 1 and NSQB > 1
                                    and skc % 2 == 1):
                                tq = mask_dma(skc // 2, 1, nc.sync)
                                for u in range(2):
                                    m_next[skc - 1 + u] = tq[:, u, :]
                            if skc >= 1 and not carry_pv and carry_norm:
                                carry_norm.pop(0)()
                        # head end: set up finishers for this head
                        assert not carry_pv and not carry_norm
                        carry_pv = [
                            lambda skc=skc_, f=emit_pv, pms=pms: f(pms[skc], skc)
                            for skc_ in range(SKT - trail, SKT)
                        ]
                        carry_norm = make_norm(
                            pv, h, ht, q0, fast=last_sqb and hi == NH - 1
                        )

                        if last_sqb and hi == 1:
                            for dc in range(D // P):
                                for half in range(2):
                                    pending.append(
                                        (
                                            lambda dc=dc, half=half, sqb=sqb: emit_oproj_piece(
                                                sqb, dc, half, [0], od3, sqb * SQB
                                            ),
                                            True,
                                        )
                                    )
                    # end heads
                    if not last_sqb:
                        for dc in range(D // P):
                            for half in range(2):
                                pending.append(
                                    (
                                        lambda sqb=sqb, dc=dc, half=half: emit_oproj_piece(
                                            sqb, dc, half, [0, 1], od3, sqb * SQB
                                        ),
                                        True,
                                    )
                                )
                # final head: flush PVs and run its norm immediately, then
                # the cc1 tail pieces, written to the odx partial
                for f in carry_pv:
                    f()
                carry_pv = []
                for fn, _ in pending:
                    fn()
                pending = []
                ev0, r0, m0, ev1, r1, m1 = carry_norm
                r0(); ev0(); r1(); ev1(); m0(); m1()
                carry_norm = []
                tail_dma = [nc.sync, nc.gpsimd]
                n = 0
                for dc in range(D // P):
                    stg = pmpool.tile([P, SQB], BF16, name="pm", tag="pm", bufs=8)
                    for half in range(2):
                        q0t = (NSQB - 1) * SQB + half * 512
                        if half:
                            po_w = ps_s_pool.tile(
                                [P, SQB], F32, name="pss", tag="pss", bufs=2
                            )
                            po = po_w[:, 0:512]
                        else:
                            po = ps_pv_pool.tile(
                                [P, 512], F32, name="po", tag="scratch", bufs=2
                            )[:]
                        nc.tensor.matmul(
                            po[:], lhsT=wo_sb[1][:, dc * P : (dc + 1) * P],
                            rhs=concat_sb[1][:, q0t : q0t + 512],
                            start=True, stop=True,
                        )
                        sl = slice(half * 512, half * 512 + 512)
                        if n % 2 == 0:
                            nc.scalar.copy(stg[:, sl], po)
                        else:
                            nc.vector.tensor_copy(stg[:, sl], po)
                        n += 1
                    tail_dma[dc % 2].dma_start(out=odx3[dc][:], in_=stg[:])

    nc.compile()
    return nc


def get_program():
    if "nc" not in _CACHE:
        _CACHE["nc"] = _build_program()
    return _CACHE["nc"]


def _split8(a):
    """fp32 array -> (fp8 hi, fp8 residual)."""
    hi = a.astype(NPFP8)
    lo = (a - hi.astype(np.float32)).astype(NPFP8)
    return hi, lo


def _pairfold(a, free):
    """[D, free] -> [128, NPAIR*2*free] partition-major DR pair layout."""
    return np.ascontiguousarray(
        a.reshape(NPAIR, 2, P, free).transpose(2, 0, 1, 3).reshape(P, NPAIR * 2 * free)
    )


def make_in_maps(q, k, v, mask, Wq, bq, Wk, bk, Wv, bv, Wo, bo, gate):
    """Host-side sharding: per-core input dict (all numpy)."""
    q, k, v = (np.asarray(a, np.float32) for a in (q, k, v))
    mask = np.asarray(mask)
    Wq, bq, Wk, bk, Wv, bv, Wo, bo, gate = (
        np.asarray(a, np.float32) for a in (Wq, bq, Wk, bk, Wv, bv, Wo, bo, gate)
    )
    scale = 1.0 / np.sqrt(HD)
    # x splits are per batch (shared by the 4 col-shards)
    xs = {}
    for b in range(B):
        for t, arr in (("q", q), ("k", k), ("v", v)):
            hi, lo = _split8(np.ascontiguousarray(arr[b].T))
            xs[(t, b, "8")] = _pairfold(hi, S)
            xs[(t, b, "l")] = _pairfold(lo, S)
        xs[("m", b)] = np.ascontiguousarray(mask[b].T).astype(NPBF16)

    in_maps = []
    for c in range(NCORES):
        b = c // (NCORES // B)
        g = c % (NCORES // B)
        cols = slice(g * COLS, (g + 1) * COLS)
        gate_cols = np.repeat(gate[g * NH : (g + 1) * NH], HD)  # [256]
        # W host-scaled x32 so fp8 residuals stay normal-range; the kernel
        # evacuates psum/32.  Score scale 1/8 folds into Wq/bq as in v1.
        w_eff = {
            "q": Wq[:, cols] * (32.0 * scale),
            "k": Wk[:, cols] * 32.0,
            "v": Wv[:, cols] * gate_cols[None, :] * 32.0,
        }
        bkc = bk[cols].astype(np.float32)
        bqc = (bq[cols] * scale).astype(np.float32)
        m = {
            "mT": xs[("m", b)],
            "wo": np.ascontiguousarray(Wo[cols, :]).astype(NPBF16),
            "biasf": np.stack(
                [bkc[0:P], bkc[P:COLS], bqc[0:P], bqc[P:COLS]], axis=1
            ),
        }
        for t in ("q", "k", "v"):
            hi, lo = _split8(w_eff[t])
            m[f"w{t}8"] = _pairfold(hi, COLS)
            m[f"w{t}l"] = _pairfold(lo, COLS)
            m[f"x{t}8"] = xs[(t, b, "8")]
            m[f"x{t}l"] = xs[(t, b, "l")]
        in_maps.append(m)
    return in_maps


LAST_RESULTS = None
OD_SCALE = 1.0 / 16.0


def kernel(q, k, v, mask, Wq, bq, Wk, bk, Wv, bv, Wo, bo, gate, trace=False):
    global LAST_RESULTS
    nc = get_program()
    in_maps = make_in_maps(q, k, v, mask, Wq, bq, Wk, bk, Wv, bv, Wo, bo, gate)
    res = run_bass_kernel_spmd(nc, in_maps, core_ids=list(range(NCORES)), trace=trace)
    LAST_RESULTS = res

    bv_ = np.asarray(bv, np.float32)
    bo_ = np.asarray(bo, np.float32)
    gate_ = np.asarray(gate, np.float32)
    Wo_ = np.asarray(Wo, np.float32)
    # attention rows sum to 1, so the bv term is a constant vector:
    # concat-level constant = repeat(gate, hd) * bv, projected through Wo.
    const = (np.repeat(gate_, HD) * bv_) @ Wo_ + bo_

    out = np.zeros((B, S, D), np.float32)
    for c in range(NCORES):
        b = c // (NCORES // B)
        # concat is 16x-scaled on-chip; undo here
        out[b] += res.results[c]["od"].astype(np.float32).T * (1.0 / 16.0)
        out[b, (NSQB - 1) * SQB :, :] += (
            res.results[c]["odx"].astype(np.float32).T * (1.0 / 16.0)
        )
    out += const[None, None, :]
    return out


# revision 44
# speedup vs baseline: 1.0016x; 1.0016x over previous
"""Multi-head attention on 8 Trainium2 NeuronCores.

Problem: B=2, S=2048, D=1024, H=16 heads (head_dim 64), boolean mask,
per-head gate, QKV/out linear projections.

Sharding: core c handles batch b=c//4 and heads 4*(c%4)..4*(c%4)+3.
Each core computes its 4 heads' attention and the partial output
projection; the host sums the 4 partials per batch and adds the constant
terms (bo, and the bv/gate contribution which is constant because
attention rows sum to 1).

Design (v2): the attention core (scores/exp/mask/PV/normalize/O-proj)
stays fp16 as in v1, but the QKV projections run as 3-term fp8e4m3
DoubleRow matmul chains:  x@W ~= x8@W8 + x8@Wlo + xlo@W8, where
x = x8 + xlo and 32*W = W8 + Wlo are host-prepared residual splits
(W scaled x32 so the residual stays in fp8 normal range; the evacuation
rescales by 1/32).  DoubleRow contracts 256 rows per pass at 0.5
cycles/row, so each projection costs 0.75x its fp16 cycles while adding
<0.2% error (measured end-to-end rel err ~2.3e-3 vs the 2e-2 gate).
fp8 x tensors also halve the input DMA bytes, pulling the first exp to
~7us.  The ACT engine runs the 128 x [128,1024] exp stream exclusively;
all psum evacuations live on DVE (tensor_scalar mult+bias).  concat is
built 16x-scaled (ones column = 1/16) so nothing changes on-chip; the
host divides the od/odx partials by 16.
"""

import sys

if "/opt/trn_rl_repo" not in sys.path:
    sys.path.insert(0, "/opt/trn_rl_repo")

import numpy as np
import ml_dtypes

import concourse.bass as bass
import concourse.bacc as bacc
import concourse.mybir as mybir
import concourse.tile as tile
from concourse.bass_utils import run_bass_kernel_spmd

BF16 = mybir.dt.float16  # fp16: same speed as bf16, 3 more mantissa bits
F32 = mybir.dt.float32
FP8 = mybir.dt.float8e4
NPBF16 = np.float16
NPFP8 = ml_dtypes.float8_e4m3
DR = mybir.MatmulPerfMode.DoubleRow
MULT = mybir.AluOpType.mult
ADD = mybir.AluOpType.add

P = 128
B, S, D = 2, 2048, 1024
HEADS, HD = 16, 64
NCORES = 8
NH = HEADS // (NCORES // B)  # heads per core = 4
COLS = NH * HD               # 256 concat columns per core
NPAIR = D // 256             # 4 DoubleRow contraction pairs (256 rows each)
SKT = S // P                 # 16 key chunks
SQB = 1024                   # query block width in the attention loop
NSQB = S // SQB
WSC = 1.0 / 32.0             # evac rescale (W is host-scaled x32)

_CACHE = {}


def _build_program():
    nc = bacc.Bacc("TRN2", debug=False)

    # x tensors in partition-major DoubleRow pair layout:
    # [D,S] -> [128, 4 pairs, 2, S] flattened to [128, 8S], so each
    # tensor-variant is ONE contiguous 128-descriptor DMA.
    xdram = {}
    for t in ("q", "k", "v"):
        for v_ in ("8", "l"):
            xdram[(t, v_)] = nc.declare_dram_parameter(
                f"x{t}{v_}", [P, NPAIR * 2 * S], FP8, isOutput=False
            )
    # weights in pair layout [128, 4 pairs, 2, 256] -> [128, 2048]
    # (partition-major so one DMA is 128 contiguous descriptors)
    wdram = {}
    for t in ("q", "k", "v"):
        for v_ in ("8", "l"):
            wdram[(t, v_)] = nc.declare_dram_parameter(
                f"w{t}{v_}", [P, NPAIR * 2 * COLS], FP8, isOutput=False
            )
    mT = nc.declare_dram_parameter("mT", [S, S], BF16, isOutput=False)
    wo = nc.declare_dram_parameter("wo", [COLS, D], BF16, isOutput=False)
    biasf = nc.declare_dram_parameter("biasf", [P, 4], F32, isOutput=False)
    od = nc.declare_dram_parameter("od", [D, S], BF16, isOutput=True)
    # cc1 (heads 2,3) partial of the LAST sq block, summed on host
    odx = nc.declare_dram_parameter("odx", [D, SQB], BF16, isOutput=True)

    xdr3 = {k: v[:] for k, v in xdram.items()}
    wdr3 = {k: v[:] for k, v in wdram.items()}
    mT3 = mT[:].rearrange("(n p) s -> n p s", p=P)
    wo3 = wo[:].rearrange("(n p) d -> n p d", p=P)
    wo3b = wo[:].rearrange("(n p) d -> p n d", p=P)
    od3 = od[:].rearrange("(n p) s -> n p s", p=P)
    odx3 = odx[:].rearrange("(n p) s -> n p s", p=P)

    def dr2(ap):
        # [p, 2*n] tile view -> [p, 2, n] for DoubleRow operands
        return ap.rearrange("p (two n) -> p two n", two=2)

    with tile.TileContext(nc) as tc:
        with (
            tc.tile_pool(name="wpool", bufs=1) as wpool,
            tc.tile_pool(name="qkpool", bufs=1) as qkpool,
            tc.tile_pool(name="vpool", bufs=1) as vpool,
            tc.tile_pool(name="maskpool", bufs=1) as maskpool,
            tc.tile_pool(name="cpool", bufs=1) as cpool,
            tc.tile_pool(name="xpool", bufs=1) as xpool,
            tc.tile_pool(name="pmpool", bufs=1) as pmpool,
            tc.tile_pool(name="npool", bufs=1) as npool,
            tc.tile_pool(name="opool", bufs=1) as opool,
        ):
            # ---------------- DMA issue (t=0) ----------------------------
            # Every DMA's transfer serializes on its issuing queue at
            # ~0.385ns/byte/partition (+ ~1us fixed), so the ramp is pure
            # pipe scheduling.  x tensors arrive column-sliced (K per
            # 512-slice, Q/V per S-half) so the first exp only waits for
            # ~36KB/partition instead of the full 96KB.
            w_sb = {}
            xparts = {}

            def wdma(t, v_, eng):
                tl = wpool.tile([P, NPAIR, 2 * COLS], FP8, name=f"w{t}{v_}")
                eng.dma_start(out=tl[:], in_=wdr3[(t, v_)])
                w_sb[(t, v_)] = [dr2(tl[:, i, :]) for i in range(NPAIR)]

            def xdma(t, v_, c0, c1, eng):
                # logical x columns [c0:c1) of all pairs, as its own tile
                w = c1 - c0
                tl = xpool.tile(
                    [P, NPAIR, 2, w], FP8, name=f"x{t}{v_}", tag=f"x{t}{v_}{c0}",
                    bufs=1,
                )
                eng.dma_start(
                    out=tl[:],
                    in_=xdr3[(t, v_)].rearrange(
                        "p (n two s) -> p n two s", n=NPAIR, two=2
                    )[:, :, :, c0:c1],
                )
                xparts.setdefault((t, v_), []).append((tl, c0, c1))

            def xap(t, v_, i, c0, c1):
                for tl, a_, b_ in xparts[(t, v_)]:
                    if a_ <= c0 and c1 <= b_:
                        return tl[:, i, :, c0 - a_ : c1 - a_]
                raise KeyError((t, v_, c0, c1))

            # mask pairs: [128, 2 skc, SQB] fp16 tiles, one DMA each
            mT2 = mT[:].rearrange("(j u p) s -> j p u s", u=2, p=P)

            def mask_dma(j, sqb, eng):
                t = maskpool.tile(
                    [P, 2, SQB], BF16, name=f"mp{j}", tag=f"mp{j}", bufs=1
                )
                eng.dma_start(
                    out=t[:], in_=mT2[j][:, :, sqb * SQB : (sqb + 1) * SQB]
                )
                return t

            # SP pipe
            xdma("k", "8", 0, 512, nc.sync)
            wdma("q", "8", nc.sync)
            wdma("q", "l", nc.sync)
            bias_t = wpool.tile([P, 4], F32, name="bias_t")
            nc.sync.dma_start(out=bias_t[:], in_=biasf[:])
            b_sb = {
                ("bk", 0): bias_t[:, 0:1],
                ("bk", 1): bias_t[:, 1:2],
                ("bq", 0): bias_t[:, 2:3],
                ("bq", 1): bias_t[:, 3:4],
            }
            xdma("q", "l", 0, SQB, nc.sync)
            mp_sb = [None] * 8
            mp_sb[1] = mask_dma(1, 0, nc.sync)
            xdma("k", "8", 512, 1024, nc.sync)
            mp_sb[2] = mask_dma(2, 0, nc.sync)
            xdma("k", "8", 1024, 1536, nc.sync)
            xdma("k", "8", 1536, 2048, nc.sync)
            xdma("v", "l", 0, SQB, nc.sync)
            mp_sb[4] = mask_dma(4, 0, nc.sync)
            xdma("v", "l", SQB, S, nc.sync)
            mp_sb[6] = mask_dma(6, 0, nc.sync)
            xdma("q", "8", SQB, S, nc.sync)
            xdma("q", "l", SQB, S, nc.sync)
            # ACT pipe (the implicit act-table load runs first; it warms
            # the exp table)
            wdma("k", "8", nc.scalar)
            wdma("k", "l", nc.scalar)
            xdma("q", "8", 0, SQB, nc.scalar)
            mp_sb[0] = mask_dma(0, 0, nc.scalar)
            # Pool pipe
            xdma("k", "l", 0, 512, nc.gpsimd)
            xdma("k", "l", 512, 1024, nc.gpsimd)
            xdma("k", "l", 1024, 1536, nc.gpsimd)
            xdma("k", "l", 1536, 2048, nc.gpsimd)
            wdma("v", "8", nc.gpsimd)
            wdma("v", "l", nc.gpsimd)
            mp_sb[3] = mask_dma(3, 0, nc.gpsimd)
            xdma("v", "8", 0, SQB, nc.gpsimd)
            mp_sb[5] = mask_dma(5, 0, nc.gpsimd)
            xdma("v", "8", SQB, S, nc.gpsimd)
            mp_sb[7] = mask_dma(7, 0, nc.gpsimd)
            m_sb = [mp_sb[i // 2][:, i % 2, :] for i in range(SKT)]

            # wo follows on SP (needed at ~85us)
            wo_all = wpool.tile([P, COLS // P, D], BF16, name="wo_all")
            nc.sync.dma_start(out=wo_all[:], in_=wo3b)
            wo_sb = [wo_all[:, i, :] for i in range(COLS // P)]

            # concat^T (normalized attention outputs, 16x-scaled, head-major)
            concat_sb = [
                cpool.tile([P, S], BF16, name=f"concat_sb{i}")
                for i in range(COLS // P)
            ]

            # ---------------- projections + attention --------------------
            # qh/kh are built from [128,512] S-slice pieces, each a
            # self-contained 12-matmul 3-term DoubleRow chain + DVE evac
            # (fp16 qh = psum/32 + bias).  Only K slice 0 and Q slices 0/1
            # run before attention; everything else (incl. the c1 halves
            # for heads 2/3) drains as pending pieces into h0/h1 slack.
            qhT_sb = {}
            for tname in ("k", "q"):
                for ch in range(2):
                    qhT_sb[(tname, ch)] = qkpool.tile(
                        [P, S], BF16, name=f"{tname}hT{ch}"
                    )

            def chain_terms(t):
                return (
                    (w_sb[(t, "8")], "8"),
                    (w_sb[(t, "l")], "8"),
                    (w_sb[(t, "8")], "l"),
                )

            vh_sb = [None] * SKT

            with (
                tc.tile_pool(name="ps_s", bufs=1, space="PSUM") as ps_s_pool,
                tc.tile_pool(name="ps_pv", bufs=1, space="PSUM") as ps_pv_pool,
            ):
                def emit_vproj(skt):
                    # one V-proj piece: vh[skt] = (xv @ Wv)[skt block]/32,
                    # 3-term DR chain; ones column = 1/16 (16x concat scale)
                    psv = ps_pv_pool.tile(
                        [P, 512], F32, name="psv", tag="scratch", bufs=2
                    )
                    n = 0
                    for w_list, xv_ in chain_terms("v"):
                        for i in range(NPAIR):
                            nc.tensor.matmul(
                                psv[:, 0:COLS],
                                lhsT=xap("v", xv_, i, skt * P, (skt + 1) * P),
                                rhs=w_list[i][:],
                                start=(n == 0),
                                stop=(n == 3 * NPAIR - 1),
                                perf_mode=DR,
                            )
                            n += 1
                    vt = vpool.tile([P, NH, HD + 1], BF16, name=f"vh_sb{skt}")
                    nc.vector.tensor_scalar(
                        vt[:, :, 0:HD],
                        psv[:, 0:COLS].rearrange("p (h d) -> p h d", h=NH),
                        WSC, None, MULT,
                    )
                    nc.gpsimd.memset(vt[:, :, HD], 1.0 / 16.0)
                    vh_sb[skt] = vt

                def emit_oproj_split(sqb, dc, half, dest3, dq0):
                    # the two cc-matmuls of an O-proj piece as separate
                    # closures so one drains per slot (keeps every sqb1
                    # slot's PE load above the exp period)
                    q0 = sqb * SQB + half * 512
                    cell = {}

                    def part0():
                        po = ps_pv_pool.tile(
                            [P, 512], F32, name="po", tag="scratch", bufs=2
                        )
                        cell["po"] = po
                        nc.tensor.matmul(
                            po[:],
                            lhsT=wo_sb[0][:, dc * P : (dc + 1) * P],
                            rhs=concat_sb[0][:, q0 : q0 + 512],
                            start=True,
                            stop=False,
                        )

                    def part1():
                        po = cell["po"]
                        nc.tensor.matmul(
                            po[:],
                            lhsT=wo_sb[1][:, dc * P : (dc + 1) * P],
                            rhs=concat_sb[1][:, q0 : q0 + 512],
                            start=False,
                            stop=True,
                        )
                        oev = opool.tile([P, 512], BF16, name="oev", tag="oev", bufs=6)
                        nc.vector.tensor_copy(oev[:], po[:])
                        nc.sync.dma_start(
                            out=dest3[dc][:, dq0 + half * 512 : dq0 + half * 512 + 512],
                            in_=oev[:],
                        )

                    return part0, part1

                def emit_oproj_piece(sqb, dc, half, ccs, dest3, dq0, evac="pool", dma=None, pool=None, rows=None):
                    # po[128d, 512sq] = sum_cc wo_cc^T @ concat_cc  (fp16)
                    q0 = sqb * SQB + half * 512
                    r0_, r1_ = rows if rows is not None else (0, P)
                    if pool == "pss":
                        po_w = ps_s_pool.tile([P, SQB], F32, name="pss", tag="pss", bufs=2)
                        po = po_w[:, 0:512]
                    else:
                        po = ps_pv_pool.tile(
                            [P, 512], F32, name="po", tag="scratch", bufs=2
                        )[:]
                    for j, cc in enumerate(ccs):
                        nc.tensor.matmul(
                            po[:],
                            lhsT=wo_sb[cc][r0_:r1_, dc * P : (dc + 1) * P],
                            rhs=concat_sb[cc][r0_:r1_, q0 : q0 + 512],
                            start=(j == 0),
                            stop=(j == len(ccs) - 1),
                        )
                    oev = opool.tile([P, 512], BF16, name="oev", tag="oev", bufs=6)
                    if evac == "act":
                        nc.scalar.copy(oev[:], po)
                    else:
                        nc.vector.tensor_copy(oev[:], po)
                    (dma or nc.sync).dma_start(
                        out=dest3[dc][:, dq0 + half * 512 : dq0 + half * 512 + 512],
                        in_=oev[:],
                    )

                ones64 = npool.tile([HD + 1, HD], BF16, name="ones64", tag="ones64", bufs=1)
                nc.vector.memset(ones64[:], 1.0)

                # deferred PE pieces, drained one per skc iteration.
                pending = []

                def drain(n):
                    k = 0
                    while pending and k < n:
                        fn, needs_concat = pending[0]
                        if needs_concat and carry_norm:
                            break
                        pending.pop(0)
                        fn()
                        k += 1

                def emit_proj_piece(tname, ch, sb):
                    # one [128,512] S-slice of a K/Q projection half: a
                    # 12-matmul 3-term DR chain into a scratch psum slot
                    pst = ps_pv_pool.tile(
                        [P, 512], F32, name="pc1", tag="scratch", bufs=2
                    )
                    n = 0
                    for w_list, xv_ in chain_terms(tname):
                        for i in range(NPAIR):
                            nc.tensor.matmul(
                                pst[:],
                                lhsT=w_list[i][:, :, ch * P : (ch + 1) * P],
                                rhs=xap(tname, xv_, i, sb * 512, (sb + 1) * 512),
                                start=(n == 0),
                                stop=(n == 3 * NPAIR - 1),
                                perf_mode=DR,
                            )
                            n += 1
                    nc.vector.tensor_scalar(
                        qhT_sb[(tname, ch)][:, sb * 512 : (sb + 1) * 512],
                        pst[:], WSC, b_sb[(f"b{tname}", ch)], MULT, ADD,
                    )

                # ramp: K slice 0 + Q slices 0/1 inline, rest pending in
                # first-use order (h0 reads kT0 skc-progressively, h2 at
                # slot 32 reads qT1/kT1, sqb1 from slot 64 reads qT0 s2/3)
                emit_proj_piece("k", 0, 0)
                emit_proj_piece("q", 0, 0)
                emit_proj_piece("q", 0, 1)
                for tname, ch, sb in (
                    ("k", 0, 1), ("k", 0, 2), ("k", 0, 3),
                    ("q", 1, 0), ("q", 1, 1),
                    ("k", 1, 0), ("k", 1, 1), ("k", 1, 2), ("k", 1, 3),
                    ("q", 0, 2), ("q", 0, 3),
                    ("q", 1, 2), ("q", 1, 3),
                ):
                    pending.append(
                        (lambda t=tname, c=ch, s=sb: emit_proj_piece(t, c, s), False)
                    )

                def make_norm(pv, h, ht, q0, fast=False):
                    # normalization for head h's pv accumulator, cut into 5
                    # pieces spread over the next head's early iterations.
                    # denom row 64 = sum(pm)/16 -> recip gives the 16x
                    # concat scale for free.
                    dnc = npool.tile(
                        [HD + 1, SQB], BF16, name="dnc", tag="dnc", bufs=2
                    )

                    def evac_half(i):
                        def fn():
                            nc.vector.tensor_copy(
                                dnc[:, i * 512 : (i + 1) * 512], pv[i][:]
                            )
                        return fn

                    def evac():
                        evac_half(0)()
                        evac_half(1)()

                    rbs = {}

                    def recip_piece(half):
                        def fn():
                            sl = slice(half * 512, half * 512 + 512)
                            if fast:
                                with nc.allow_low_precision("fp16 softmax denom"):
                                    nc.vector.reciprocal(
                                        out=dnc[HD : HD + 1, sl],
                                        in_=pv[half][HD : HD + 1, :],
                                    )
                                rb = ps_pv_pool.tile(
                                    [P, 512], F32, name="rbf", tag="scratch", bufs=2
                                )
                                nc.tensor.matmul(
                                    rb[0:HD, :],
                                    lhsT=ones64[HD : HD + 1, :],
                                    rhs=dnc[HD : HD + 1, sl],
                                    start=True,
                                    stop=True,
                                )
                                rbs[half] = rb[0:HD, :]
                                return
                            dn0 = npool.tile([1, 512], BF16, name="dn0", tag="dn0", bufs=1)
                            nc.sync.dma_start(out=dn0[:], in_=dnc[HD : HD + 1, sl])
                            with nc.allow_low_precision("fp16 softmax denom"):
                                nc.vector.reciprocal(out=dn0[:], in_=dn0[:])
                            rb = npool.tile(
                                [HD, 512], BF16, name=f"rb{half}", tag=f"rb{half}", bufs=1
                            )
                            nc.gpsimd.partition_broadcast(rb[:], dn0[:])
                            rbs[half] = rb[:]
                        return fn

                    def mul_piece(half):
                        def fn():
                            sl = slice(half * 512, half * 512 + 512)
                            cs = slice(q0 + half * 512, q0 + half * 512 + 512)
                            if h % 2 == 0:
                                nc.vector.tensor_mul(
                                    concat_sb[ht][0:HD, cs], dnc[0:HD, sl], rbs[half]
                                )
                            else:
                                tmp = npool.tile([HD, 512], BF16, name="tmpn", tag="tmpn", bufs=1)
                                nc.vector.tensor_mul(tmp[:], dnc[0:HD, sl], rbs[half])
                                nc.sync.dma_start(out=concat_sb[ht][HD:P, cs], in_=tmp[:])
                        return fn

                    if fast:
                        return [
                            evac_half(0), recip_piece(0), mul_piece(0),
                            evac_half(1), recip_piece(1), mul_piece(1),
                        ]
                    return [evac, recip_piece(0), mul_piece(0), recip_piece(1), mul_piece(1)]

                carry_pv = []
                carry_norm = []

                m_cur = m_sb
                m_next = [None] * SKT
                for sqb in range(NSQB):
                    q0 = sqb * SQB
                    last_sqb = sqb == NSQB - 1
                    if sqb == 1:
                        m_cur = m_next
                    heads = [0, 1, 3, 2] if last_sqb else [0, 1, 2, 3]
                    for hi, h in enumerate(heads):
                        ht, hp = h // 2, HD * (h % 2)
                        qT = qhT_sb[("q", ht)]
                        kT = qhT_sb[("k", ht)]
                        pv = [
                            ps_pv_pool.tile(
                                [HD + 1, 512], F32, name=f"pspv{i}", tag=f"pspv{i}", bufs=1
                            )
                            for i in range(2)
                        ]

                        def emit_pv(pm_t, skc, pv=pv, h=h):
                            for i in range(SQB // 512):
                                nc.tensor.matmul(
                                    pv[i][:],
                                    lhsT=vh_sb[skc][:, h, :],
                                    rhs=pm_t[:, i * 512 : (i + 1) * 512],
                                    start=(skc == 0),
                                    stop=(skc == SKT - 1),
                                )

                        # h0 of sqb0 also hosts the V-proj pieces (shifted
                        # 5 slots late: the xv DMAs land ~14-21us); its PV
                        # trails by 7; V pieces 11-15 run early in h1
                        trail = 7 if (sqb == 0 and hi == 0) else 2
                        pms = [None] * SKT
                        for skc in range(SKT):
                            ss = ps_s_pool.tile(
                                [P, SQB], F32, name="pss", tag="pss", bufs=2
                            )
                            for i in range(SQB // 512):
                                nc.tensor.matmul(
                                    ss[:, i * 512 : (i + 1) * 512],
                                    lhsT=kT[hp : hp + HD, skc * P : (skc + 1) * P],
                                    rhs=qT[hp : hp + HD, q0 + i * 512 : q0 + (i + 1) * 512],
                                    start=True,
                                    stop=True,
                                )
                            if skc <= 6 and carry_pv:
                                carry_pv.pop(0)()
                            if sqb == 0 and hi == 0:
                                if skc >= 5:
                                    emit_vproj(skc - 5)
                                if skc % 4 == 1:
                                    drain(1)
                            else:
                                if sqb == 0 and hi == 1 and skc <= 4:
                                    emit_vproj(11 + skc)
                                if (
                                    (skc % 4 == 1 if sqb == 0 else skc % 2 == 1)
                                    or (last_sqb and hi >= 2 and len(pending) >= 1)
                                ):
                                    drain(1)
                            if skc >= trail:
                                emit_pv(pms[skc - trail], skc - trail)
                            pm = pmpool.tile([P, SQB], BF16, name="pm", tag="pm", bufs=8)
                            if last_sqb and hi == NH - 1 and skc == SKT - 1:
                                for eh in range(2):
                                    es = slice(eh * 512, eh * 512 + 512)
                                    nc.scalar.activation(
                                        pm[:, es], ss[:, es],
                                        mybir.ActivationFunctionType.Exp,
                                    )
                            else:
                                nc.scalar.activation(
                                    pm[:], ss[:], mybir.ActivationFunctionType.Exp
                                )
                            meng = nc.gpsimd if skc % 8 in (3, 6) else nc.vector
                            meng.tensor_mul(pm[:], pm[:], m_cur[skc])
                            pms[skc] = pm
                            if (sqb == 0 and hi == NH - 1 and NSQB > 1
                                    and skc % 2 == 1):
                                tq = mask_dma(skc // 2, 1, nc.sync)
                                for u in range(2):
                                    m_next[skc - 1 + u] = tq[:, u, :]
                            if skc >= 1 and not carry_pv and carry_norm:
                                carry_norm.pop(0)()
                        # head end: set up finishers for this head
                        assert not carry_pv and not carry_norm
                        carry_pv = [
                            lambda skc=skc_, f=emit_pv, pms=pms: f(pms[skc], skc)
                            for skc_ in range(SKT - trail, SKT)
                        ]
                        carry_norm = make_norm(
                            pv, h, ht, q0, fast=last_sqb and hi == NH - 1
                        )

                        if last_sqb and hi == 1:
                            for dc in range(D // P):
                                for half in range(2):
                                    pending.append(
                                        (
                                            lambda dc=dc, half=half, sqb=sqb: emit_oproj_piece(
                                                sqb, dc, half, [0], od3, sqb * SQB
                                            ),
                                            True,
                                        )
                                    )
                    # end heads
                    if not last_sqb:
                        for dc in range(D // P):
                            for half in range(2):
                                pending.append(
                                    (
                                        lambda sqb=sqb, dc=dc, half=half: emit_oproj_piece(
                                            sqb, dc, half, [0, 1], od3, sqb * SQB
                                        ),
                                        True,
                                    )
                                )
                # final head: flush PVs and run its norm immediately, then
                # the cc1 tail pieces, written to the odx partial
                for f in carry_pv:
                    f()
                carry_pv = []
                for fn, _ in pending:
                    fn()
                pending = []
                ev0, r0, m0, ev1, r1, m1 = carry_norm
                r0(); ev0(); r1(); ev1(); m0(); m1()
                carry_norm = []
                tail_dma = [nc.sync, nc.gpsimd]
                n = 0
                for dc in range(D // P):
                    stg = pmpool.tile([P, SQB], BF16, name="pm", tag="pm", bufs=8)
                    for half in range(2):
                        q0t = (NSQB - 1) * SQB + half * 512
                        if half:
                            po_w = ps_s_pool.tile(
                                [P, SQB], F32, name="pss", tag="pss", bufs=2
                            )
                            po = po_w[:, 0:512]
                        else:
                            po = ps_pv_pool.tile(
                                [P, 512], F32, name="po", tag="scratch", bufs=2
                            )[:]
                        nc.tensor.matmul(
                            po[:], lhsT=wo_sb[1][:, dc * P : (dc + 1) * P],
                            rhs=concat_sb[1][:, q0t : q0t + 512],
                            start=True, stop=True,
                        )
                        sl = slice(half * 512, half * 512 + 512)
                        if n % 2 == 0:
                            nc.scalar.copy(stg[:, sl], po)
                        else:
                            nc.vector.tensor_copy(stg[:, sl], po)
                        n += 1
                    tail_dma[dc % 2].dma_start(out=odx3[dc][:], in_=stg[:])

    nc.compile()
    return nc


def get_program():
    if "nc" not in _CACHE:
        _CACHE["nc"] = _build_program()
    return _CACHE["nc"]


def _split8(a):
    """fp32 array -> (fp8 hi, fp8 residual)."""
    hi = a.astype(NPFP8)
    lo = (a - hi.astype(np.float32)).astype(NPFP8)
    return hi, lo


def _pairfold(a, free):
    """[D, free] -> [128, NPAIR*2*free] partition-major DR pair layout."""
    return np.ascontiguousarray(
        a.reshape(NPAIR, 2, P, free).transpose(2, 0, 1, 3).reshape(P, NPAIR * 2 * free)
    )


def make_in_maps(q, k, v, mask, Wq, bq, Wk, bk, Wv, bv, Wo, bo, gate):
    """Host-side sharding: per-core input dict (all numpy)."""
    q, k, v = (np.asarray(a, np.float32) for a in (q, k, v))
    mask = np.asarray(mask)
    Wq, bq, Wk, bk, Wv, bv, Wo, bo, gate = (
        np.asarray(a, np.float32) for a in (Wq, bq, Wk, bk, Wv, bv, Wo, bo, gate)
    )
    scale = 1.0 / np.sqrt(HD)
    # x splits are per batch (shared by the 4 col-shards)
    xs = {}
    for b in range(B):
        for t, arr in (("q", q), ("k", k), ("v", v)):
            hi, lo = _split8(np.ascontiguousarray(arr[b].T))
            xs[(t, b, "8")] = _pairfold(hi, S)
            xs[(t, b, "l")] = _pairfold(lo, S)
        xs[("m", b)] = np.ascontiguousarray(mask[b].T).astype(NPBF16)

    in_maps = []
    for c in range(NCORES):
        b = c // (NCORES // B)
        g = c % (NCORES // B)
        cols = slice(g * COLS, (g + 1) * COLS)
        gate_cols = np.repeat(gate[g * NH : (g + 1) * NH], HD)  # [256]
        # W host-scaled x32 so fp8 residuals stay normal-range; the kernel
        # evacuates psum/32.  Score scale 1/8 folds into Wq/bq as in v1.
        w_eff = {
            "q": Wq[:, cols] * (32.0 * scale),
            "k": Wk[:, cols] * 32.0,
            "v": Wv[:, cols] * gate_cols[None, :] * 32.0,
        }
        bkc = bk[cols].astype(np.float32)
        bqc = (bq[cols] * scale).astype(np.float32)
        m = {
            "mT": xs[("m", b)],
            "wo": np.ascontiguousarray(Wo[cols, :]).astype(NPBF16),
            "biasf": np.stack(
                [bkc[0:P], bkc[P:COLS], bqc[0:P], bqc[P:COLS]], axis=1
            ),
        }
        for t in ("q", "k", "v"):
            hi, lo = _split8(w_eff[t])
            m[f"w{t}8"] = _pairfold(hi, COLS)
            m[f"w{t}l"] = _pairfold(lo, COLS)
            m[f"x{t}8"] = xs[(t, b, "8")]
            m[f"x{t}l"] = xs[(t, b, "l")]
        in_maps.append(m)
    return in_maps


LAST_RESULTS = None
OD_SCALE = 1.0 / 16.0


def kernel(q, k, v, mask, Wq, bq, Wk, bk, Wv, bv, Wo, bo, gate, trace=False):
    global LAST_RESULTS
    nc = get_program()
    in_maps = make_in_maps(q, k, v, mask, Wq, bq, Wk, bk, Wv, bv, Wo, bo, gate)
    res = run_bass_kernel_spmd(nc, in_maps, core_ids=list(range(NCORES)), trace=trace)
    LAST_RESULTS = res

    bv_ = np.asarray(bv, np.float32)
    bo_ = np.asarray(bo, np.float32)
    gate_ = np.asarray(gate, np.float32)
    Wo_ = np.asarray(Wo, np.float32)
    # attention rows sum to 1, so the bv term is a constant vector:
    # concat-level constant = repeat(gate, hd) * bv, projected through Wo.
    const = (np.repeat(gate_, HD) * bv_) @ Wo_ + bo_

    out = np.zeros((B, S, D), np.float32)
    for c in range(NCORES):
        b = c // (NCORES // B)
        # concat is 16x-scaled on-chip; undo here
        out[b] += res.results[c]["od"].astype(np.float32).T * (1.0 / 16.0)
        out[b, (NSQB - 1) * SQB :, :] += (
            res.results[c]["odx"].astype(np.float32).T * (1.0 / 16.0)
        )
    out += const[None, None, :]
    return out
